# revision 44
# baseline (speedup 1.0000x reference)
"""Trainium2 Bass kernel for nn_Attention_8083128451525 (sparse_attention).

Strategy (validated against reference by golden-model + HW probes):
  - data-parallel: core b computes batch element b (B=8, 8 cores), no collectives
  - all matmuls in float32r (tf32-like, 1 cycle/row at N>=256), fp32 storage
  - 2D rope applied via stream_shuffle (pair swap) + 2 muls + 2 half-adds
  - decomposed rel-pos bias folded into ONE K=128 augmented S^T matmul:
      K~ = [roped k | onehot_h*8 | onehot_w*8],  Q~ = [roped q | U^T | V^T]
    where U[i,h'] = rq[i].rel_pos_h[h_i-h'+31], built by a P = RhT.T @ rq
    matmul + DRAM-bounce gather-DMA with overlapping access pattern.
    exp uses scale=1/8 so onehots are pre-scaled by 8.
  - softmax denominator = ones-column folded into augmented V (PV matmul
    row 64); normalization via reciprocal + 0-stride DMA broadcast.
  - qkv/proj biases folded as K=1 ones-row matmuls or ACT per-partition bias.
"""

import os
import sys

for _p in ("/opt/trn_rl_repo", "/root/.axon_site/_ro/trn_rl_repo"):
    if os.path.isdir(_p) and _p not in sys.path:
        sys.path.insert(0, _p)

import json
from contextlib import ExitStack

import numpy as np

import bass_rust
import concourse.bass as bass
import concourse.tile as tile
from concourse import mybir
from concourse.bass_utils import run_bass_kernel_spmd

F32R = mybir.dt.float32r
F32 = mybir.dt.float32
BF16 = mybir.dt.bfloat16
AF = mybir.ActivationFunctionType

NH, HD, HH, WW = 12, 64, 32, 32
L = HH * WW          # 1024
DIM = NH * HD        # 768
ROPE_THETA = 10000.0

# ---------------------------------------------------------------------------
# BIR post-fix: hoist excess semaphore waits onto injected NoOps (walrus
# instruction structs have limited wait slots; f32r Matmult rejects >1).
# ---------------------------------------------------------------------------
_nop_counter = [0]


def _transform_bir(json_bytes: bytes) -> bytes:
    data = json.loads(json_bytes)
    for fn in data.get("functions", []):
        for blk in fn.get("blocks", []):
            out = []
            for ins in blk.get("instructions", []):
                si = ins.get("sync_info")
                waits = (si or {}).get("on_wait") or []
                if len(waits) > 1:
                    keep = waits[-1:]
                    for w in waits[:-1]:
                        _nop_counter[0] += 1
                        out.append({
                            "name": f"I-birfix-{_nop_counter[0]}",
                            "opcode": "NoOp",
                            "engine": ins.get("engine"),
                            "ins": [],
                            "outs": [],
                            "sync_info": {"on_wait": [w], "on_update": []},
                        })
                    si["on_wait"] = keep
                out.append(ins)
            blk["instructions"] = out
    return json.dumps(data).encode()


def _install_birfix():
    if getattr(bass.Bass, "_birfix_installed", False):
        return
    orig = bass.Bass.to_json_bytes

    def patched(self, *a, **kw):
        return _transform_bir(orig(self, *a, **kw))

    bass.Bass.to_json_bytes = patched
    bass.Bass._birfix_installed = True


_install_birfix()


def _vp(pairs):
    return bass_rust.VecI64Pair(pairs)


def _cap(ap, pairs, offset=None):
    ap = ap.copy()
    ap.ap = _vp(pairs)
    if offset is not None:
        ap.offset = offset
    return ap


# ---------------------------------------------------------------------------
# Host-side constant prep
# ---------------------------------------------------------------------------
def _rope_angles():
    f = 1.0 / (ROPE_THETA ** (np.arange(0, HD, 4)[: HD // 4].astype(np.float32) / HD))
    t = np.arange(L)
    tx = (t % WW).astype(np.float32)
    ty = (t // WW).astype(np.float32)
    return np.concatenate([np.outer(tx, f), np.outer(ty, f)], axis=-1)  # (L, 32)


def _host_prep(qkv_w, qkv_b, proj_w, proj_b, rel_pos_h, rel_pos_w):
    cols, bias = [], []
    for n in range(NH):
        cols.append(qkv_w[0 * DIM + n * HD : 0 * DIM + (n + 1) * HD])
        cols.append(qkv_w[1 * DIM + n * HD : 1 * DIM + (n + 1) * HD])
        bias.append(qkv_b[0 * DIM + n * HD : 0 * DIM + (n + 1) * HD])
        bias.append(qkv_b[1 * DIM + n * HD : 1 * DIM + (n + 1) * HD])
    W_t = np.ascontiguousarray(np.concatenate(cols, axis=0).T)        # (768, 1536)
    b_qk = np.concatenate(bias).reshape(NH, 128).T.copy()             # (128, 12)

    Wv_t = np.ascontiguousarray(qkv_w[2 * DIM :].T)                   # (768, 768)
    bv_row = qkv_b[2 * DIM :].reshape(1, DIM).copy()
    P_t = np.ascontiguousarray(proj_w.T)                              # (768, 768)
    pbT = np.ascontiguousarray(proj_b.reshape(6, 128).T)              # (128, 6)

    ang = _rope_angles()
    cos, sin = np.cos(ang), np.sin(ang)                               # (L, 32)
    CC = np.empty((128, L), np.float32)
    SS = np.empty((128, L), np.float32)
    for p in range(128):
        m = (p % 64) // 2
        CC[p] = cos[:, m]
        SS[p] = sin[:, m] if (p % 2) else -sin[:, m]

    j = np.arange(L)
    OHOW = np.zeros((64, L), np.float32)
    for p in range(32):
        OHOW[p] = 8.0 * ((j >> 5) == (31 - p))
        OHOW[32 + p] = 8.0 * ((j & 31) == (31 - p))

    # padded to 64 output rows (M=64) so downstream copies stay aligned
    RhT = np.zeros((HD, 64), np.float32)
    RhT[:, 0:63] = rel_pos_h.T
    RwT = np.zeros((HD, 64), np.float32)
    RwT[:, 0:63] = rel_pos_w.T

    import ml_dtypes as _mld
    CC = CC.astype(_mld.bfloat16)
    SS = SS.astype(_mld.bfloat16)
    ones_row = np.ones((1, 128), np.float32)

    return dict(W_t=W_t, b_qk=b_qk, Wv_t=Wv_t, bv_row=bv_row, P_t=P_t,
                pbT=pbT, CC=CC, SS=SS, OHOW=OHOW, RhT=RhT, RwT=RwT,
                ones_row=ones_row)


# ---------------------------------------------------------------------------
# Bass program (one core, one batch element)
# ---------------------------------------------------------------------------
def build_bass(iters=1, gps=False, ebf16=False, allbf16=False, pv2=True,
               sbf16=False, ts2=True, t22=False, deep=True, wq5=True,
               probe=(), ppse="act", ri_pool=False, pp_split=False, skew=False, psalt=False,
               bskew=False, pj_alt=False):
    probe = set(probe)
    if allbf16:
        ebf16 = True
    nc = bass.Bass()

    DT = BF16 if allbf16 else F32R
    SDT = BF16 if (sbf16 or allbf16) else F32R
    xT = nc.declare_dram_parameter("xT", [DIM, L], DT, isOutput=False)
    W_t = nc.declare_dram_parameter("W_t", [DIM, 1536], DT, isOutput=False)
    b_qk = nc.declare_dram_parameter("b_qk", [128, NH], F32, isOutput=False)
    Wv_t = nc.declare_dram_parameter("Wv_t", [DIM, DIM], DT, isOutput=False)
    bv_row = nc.declare_dram_parameter("bv_row", [1, DIM], DT, isOutput=False)
    P_t = nc.declare_dram_parameter("P_t", [DIM, DIM], DT, isOutput=False)
    pbT_d = nc.declare_dram_parameter("pbT", [128, 6], F32, isOutput=False)
    CCd = nc.declare_dram_parameter("CC", [128, L], BF16, isOutput=False)
    SSd = nc.declare_dram_parameter("SS", [128, L], BF16, isOutput=False)
    OHOWd = nc.declare_dram_parameter("OHOW", [64, L], SDT, isOutput=False)
    RhTd = nc.declare_dram_parameter("RhT", [HD, 64], SDT, isOutput=False)
    RwTd = nc.declare_dram_parameter("RwT", [HD, 64], SDT, isOutput=False)
    ones_d = nc.declare_dram_parameter("ones_row", [1, 128], DT, isOutput=False)
    outD = nc.declare_dram_parameter("out", [DIM, L], F32R, isOutput=True)

    with tile.TileContext(nc) as tc:
        est = ExitStack()
        consts = est.enter_context(tc.tile_pool(name="consts", bufs=1))
        xtp = est.enter_context(tc.tile_pool(name="xtp", bufs=1))
        wqp = est.enter_context(tc.tile_pool(name="wqp", bufs=(5 if wq5 else 4) if deep else 3))
        vwp = est.enter_context(tc.tile_pool(name="vwp", bufs=1))
        ripool = est.enter_context(tc.tile_pool(name="ripool", bufs=2))
        vap = est.enter_context(tc.tile_pool(name="vap", bufs=1))
        qap = est.enter_context(tc.tile_pool(name="qap", bufs=3))
        kap = est.enter_context(tc.tile_pool(name="kap", bufs=3))
        scr = est.enter_context(tc.tile_pool(name="scr", bufs=1))
        tsp = est.enter_context(tc.tile_pool(name="tsp", bufs=2 if ts2 else 1))
        t2p = est.enter_context(tc.tile_pool(name="t2p", bufs=2 if t22 else 1))
        pps = est.enter_context(tc.tile_pool(name="pps", bufs=2))
        epool = est.enter_context(tc.tile_pool(name="epool", bufs=4 if deep else 3))
        misc = est.enter_context(tc.tile_pool(name="misc", bufs=2))
        recp = est.enter_context(tc.tile_pool(name="recp", bufs=1))
        outtp = est.enter_context(tc.tile_pool(name="outtp", bufs=1))
        osb = est.enter_context(tc.tile_pool(name="osb", bufs=1))
        dram = est.enter_context(tc.tile_pool(name="dram", bufs=6 if ts2 else 3, space="DRAM"))
        psum = est.enter_context(tc.tile_pool(name="psum", bufs=1, space="PSUM"))
        psum_s = est.enter_context(tc.tile_pool(name="psum_s", bufs=2, space="PSUM"))
        psum_pv = est.enter_context(tc.tile_pool(name="psum_pv", bufs=1, space="PSUM"))
        pvsp = est.enter_context(tc.tile_pool(name="pvsp", bufs=2))

        # ---- constants ----
        cc = consts.tile([128, L], BF16, tag="cc")
        ss = consts.tile([128, L], BF16, tag="ss")
        bqk = consts.tile([128, NH], F32, tag="bqk")
        bv = consts.tile([1, DIM], DT, tag="bv")
        pbT = consts.tile([128, 6], F32, tag="pbT")
        rht = consts.tile([HD, 64], SDT, tag="rht")
        rwt = consts.tile([HD, 64], SDT, tag="rwt")
        ones1 = consts.tile([1, 128], DT, tag="ones1")
        ohowc = consts.tile([64, L], SDT, tag="ohowc")
        nc.scalar.dma_start(cc[:], CCd[:])
        nc.scalar.dma_start(ss[:], SSd[:])
        nc.scalar.dma_start(bqk[:], b_qk[:])
        nc.scalar.dma_start(bv[:], bv_row[:])
        nc.scalar.dma_start(pbT[:], pbT_d[:])
        nc.scalar.dma_start(rht[:], RhTd[:])
        nc.scalar.dma_start(rwt[:], RwTd[:])
        nc.scalar.dma_start(ones1[:], ones_d[:])
        nc.scalar.dma_start(ohowc[:], OHOWd[:])

        # ---- xT as one wide tile (one DMA), chunk views per k ----
        xts = []
        for k in range(6):
            t = xtp.tile([128, L], DT, tag=f"xt{k}")
            nc.scalar.dma_start(t[:], xT[k * 128 : (k + 1) * 128, :])
            xts.append(t)

        for _it in range(iters):
            # ---- per-head pipeline state (prep defined below, invoked early) ----
            outt = []
            for k in range(6):
                ot = outtp.tile([128, L], DT, tag=f"ot{k}", name=f"ot{k}")
                outt.append(ot)
            swap_mask = [i ^ 1 for i in range(32)]
            vaug = []

            # ---- per-head pipeline ----
            def prep(n):
                # qk matmul for head n: W m-tile n = [q_n | k_n] columns
                qa = qap.tile([128, L], SDT, tag="qa", name="qa")
                ka = kap.tile([128, L], SDT, tag="ka", name="ka")
                if "no_qk" not in probe:
                    wm = wqp.tile([128, 768], DT, tag="wm", name="wm")
                    nc.sync.dma_start(
                        wm[:], _cap(W_t[:], [[1536, 128], [128 * 1536, 6], [1, 128]], n * 128))
                    qkps = psum.tile([128, L], F32, tag="mm", name="qkps")
                    for k in range(6):
                        for ih in range(2):
                            nc.tensor.matmul(
                                qkps[:, ih * 512 : (ih + 1) * 512],
                                wm[:, k * 128 : (k + 1) * 128],
                                xts[k][:, ih * 512 : (ih + 1) * 512],
                                start=(k == 0), stop=(k == 5),
                            )
                    ri = ripool.tile([128, L], F32, tag="ri", name="ri")
                    if ri_pool:
                        nc.gpsimd.tensor_scalar_add(ri[:], qkps[:], bqk[:, n : n + 1])
                    else:
                        nc.scalar.activation(ri[:], qkps[:], AF.Identity,
                                             bias=bqk[:, n : n + 1], scale=1.0)

                if "no_rope" in probe or "no_qk" in probe:
                    nc.vector.tensor_copy(qa[0:64, :], cc[0:64, :])
                    nc.vector.tensor_copy(ka[0:64, :], cc[64:128, :])
                else:
                    # rope
                    ts_ = tsp.tile([128, L], F32, tag="ts", name="ts_")
                    nc.vector.stream_shuffle(ts_[:], ri[:], swap_mask)
                    t1 = scr.tile([128, L], F32, tag="t1", name="t1")
                    nc.vector.tensor_mul(t1[:], ri[:], cc[:])
                    t2 = t2p.tile([128, L], F32, tag="t2", name="t2")
                    (nc.gpsimd if gps else nc.vector).tensor_mul(t2[:], ts_[:], ss[:])
                    nc.vector.tensor_add(qa[0:64, :], t1[0:64, :], t2[0:64, :])
                    nc.vector.tensor_add(ka[0:64, :], t1[64:128, :], t2[64:128, :])
                nc.vector.tensor_copy(ka[64:128, :], ohowc[:])

                if "no_relpos" in probe:
                    nc.vector.tensor_copy(qa[64:128, :], ss[0:64, :])
                else:
                    # rel-pos P matmuls on roped q (M padded to 64);
                    # PSUM DMA'd straight to DRAM (no act copy)
                    php = psum.tile([128, L], F32, tag="mm", name="php")
                    for ih in range(2):
                        nc.tensor.matmul(php[0:64, ih * 512 : (ih + 1) * 512],
                                         rht[:], qa[0:64, ih * 512 : (ih + 1) * 512],
                                         start=True, stop=True)

                    pwp = psum.tile([128, L], F32, tag="mm", name="pwp")
                    for ih in range(2):
                        rhs = _cap(qa[0:64, :], [[L, 64], [1, 16], [32, 32]], ih * 16)
                        nc.tensor.matmul(pwp[0:64, ih * 512 : (ih + 1) * 512],
                                         rwt[:], rhs, start=True, stop=True)

                    phs = pps.tile([64, L], SDT, tag="phs", name="phs")
                    pws = pps.tile([64, L], SDT, tag="pws", name="pws")
                    if pp_split:
                        nc.vector.tensor_copy(phs[:], php[0:64, :])
                        nc.gpsimd.tensor_copy(pws[:], pwp[0:64, :])
                    else:
                        nc.scalar.activation(phs[:], php[0:64, :], AF.Copy)
                        nc.scalar.activation(pws[:], pwp[0:64, :], AF.Copy)

                    if "no_bounce" in probe:
                        nc.vector.tensor_copy(qa[64:128, :], ss[0:64, :])
                    else:
                        phd = dram.tile([64, L], SDT, tag="phd", name="phd")
                        nc.sync.dma_start(phd[:], phs[:])
                        pwd = dram.tile([64, L], SDT, tag="pwd", name="pwd")
                        nc.sync.dma_start(pwd[:], pws[:])

                        # gather U^T into qa[64:96]; gather Vt (w-major) + unpermute
                        nc.sync.dma_start(qa[64:96, :],
                                          _cap(phd[:], [[1024, 32], [1056, 32], [1, 32]]))
                        vts = misc.tile([32, L], SDT, tag="vts", name="vts")
                        nc.sync.dma_start(vts[:],
                                          _cap(pwd[:], [[1024, 32], [1056, 32], [1, 32]]))
                        (nc.gpsimd if gps else nc.vector).tensor_copy(
                            _cap(qa[96:128, :], [[L, 32], [32, 32], [1, 32]]),
                            _cap(vts[:], [[L, 32], [1, 32], [32, 32]]))
                return qa, ka

            edt = BF16 if ebf16 else F32R
            njt = 4 if "half_jt" in probe else 8

            def s_exp(n, qa, ka, jt):
                sps = psum_s.tile([128, L], F32, tag="s", name="sps")
                for ih in range(2):
                    nc.tensor.matmul(sps[:, ih * 512 : (ih + 1) * 512],
                                     ka[:, jt * 128 : (jt + 1) * 128],
                                     qa[:, ih * 512 : (ih + 1) * 512],
                                     start=True, stop=True)
                ej = epool.tile([128, L], edt, tag="ej", name="ej")
                nc.scalar.activation(ej[:], sps[:],
                                     AF.Copy if "exp_copy" in probe else AF.Exp,
                                     scale=0.125)
                return ej

            def pv_step(n, pvp, ej, jt):
                for ih in range(2):
                    nc.tensor.matmul(pvp[:, ih * 512 : (ih + 1) * 512],
                                     vaug[jt][:, 65 * n : 65 * n + 65],
                                     ej[:, ih * 512 : (ih + 1) * 512],
                                     start=(jt == 0), stop=(jt == njt - 1))

            def finish_head(n, pvp):
                # free the PV psum bank early: copy to SBUF on DVE
                pvs = pvsp.tile([65, L], F32, tag="pvs", name="pvs")
                nc.vector.tensor_copy(pvs[:], pvp[:])
                if "no_recb" in probe:
                    nc.vector.tensor_copy(
                        outt[n // 2][(n % 2) * 64 : (n % 2) * 64 + 64, :], pvs[0:64, :])
                else:
                    # normalize via reciprocal + 0-stride DMA broadcast
                    rec = recp.tile([1, L], F32, tag="rec", name="rec")
                    nc.vector.reciprocal(rec[:], pvs[64:65, :])
                    recd = dram.tile([1, L], F32, tag="recd", name="recd")
                    nc.sync.dma_start(recd[:], rec[:])
                    rec64 = misc.tile([64, L], F32, tag="rec64", name="rec64")
                    nc.sync.dma_start(rec64[:], _cap(recd[:], [[0, 64], [1, L]]))
                    nc.vector.tensor_mul(outt[n // 2][(n % 2) * 64 : (n % 2) * 64 + 64, :],
                                         pvs[0:64, :], rec64[:])

            # heads 0/1 prep first (their W loads + qk matmuls lead the queue),
            # then the v phase, whose weight DMAs trail the early wm loads
            state = {0: prep(0), 1: prep(1)}

            vws = []
            for k in range(6):
                t = vwp.tile([128, DIM], DT, tag=f"vw{k}")
                nc.scalar.dma_start(t[:], Wv_t[k * 128 : (k + 1) * 128, :])
                vws.append(t)
            for m in range(8):
                va = vap.tile([128, 780], BF16 if ebf16 else F32R, tag=f"va{m}")
                nc.vector.memset(_cap(va[:], [[780, 128], [65, 12], [1, 64]]), 0.0)
                nc.vector.memset(_cap(va[:], [[780, 128], [65, 12]], 64), 1.0)
                if "no_v" not in probe:
                    if psalt and m % 2 == 1:
                        vps = psum.tile([128, L], F32, tag="mm", name="vps")
                    else:
                        vps = psum_s.tile([128, L], F32, tag="s", name="vps")
                    for k in range(6):
                        for c0, cw in ((0, 512), (512, 256)):
                            nc.tensor.matmul(
                                vps[:, c0 : c0 + cw],
                                xts[k][:, m * 128 : (m + 1) * 128],
                                vws[k][:, c0 : c0 + cw],
                                start=(k == 0), stop=False,
                            )
                    for c0, cw in ((0, 512), (512, 256)):
                        nc.tensor.matmul(vps[:, c0 : c0 + cw], ones1[:, 0:128],
                                         bv[:, c0 : c0 + cw], start=False, stop=True)
                    nc.scalar.activation(_cap(va[:], [[780, 128], [65, 8], [1, 64]]),
                                         vps[:, 0:512], AF.Copy)
                    nc.scalar.activation(_cap(va[:], [[780, 128], [65, 4], [1, 64]], 65 * 8),
                                         vps[:, 512:768], AF.Copy)
                vaug.append(va)

            # proj weights early: vw tags are free once the v matmuls read them
            pts = []
            for k in range(6):
                t = vwp.tile([128, DIM], DT, tag=f"vw{k}")
                nc.scalar.dma_start(t[:], P_t[k * 128 : (k + 1) * 128, :])
                pts.append(t)

            if "no_attn" in probe:
                for n in range(2, NH):
                    state[n] = prep(n)
            else:
                steps = [(n, jt) for n in range(NH) for jt in range(njt)]
                ej_q = []     # (n, pvp, ej, jt) awaiting PV
                pvps = {}
                for i, (n, jt) in enumerate(steps):
                    if jt == 0:
                        if n + 2 < NH:
                            state[n + 2] = prep(n + 2)
                        pvps[n] = psum_pv.tile([65, L], F32, tag="pv", name="pvp")
                    qa, ka = state[n]
                    depth = 1 if (skew or (bskew and jt == njt - 1)) else 0
                    ej_q.append((n, pvps[n], s_exp(n, qa, ka, jt), jt))
                    if len(ej_q) > depth:
                        pn, ppvp, pej, pjt = ej_q.pop(0)
                        pv_step(pn, ppvp, pej, pjt)
                        if pjt == njt - 1:
                            finish_head(pn, ppvp)
                            state.pop(pn)
                while ej_q:
                    pn, ppvp, pej, pjt = ej_q.pop(0)
                    pv_step(pn, ppvp, pej, pjt)
                    if pjt == njt - 1:
                        finish_head(pn, ppvp)
                        state.pop(pn)

            # ---- proj (transposed orientation: out^T[d, i]) ----
            if "no_proj" in probe:
                continue
            for dt_ in range(6):
                if (psalt or pj_alt) and dt_ % 2 == 1:
                    prp = psum.tile([128, L], F32, tag="mm", name="prp")
                else:
                    prp = psum_s.tile([128, L], F32, tag="s", name="prp")
                for k in range(6):
                    for ih in range(2):
                        nc.tensor.matmul(
                            prp[:, ih * 512 : (ih + 1) * 512],
                            pts[k][:, dt_ * 128 : (dt_ + 1) * 128],
                            outt[k][:, ih * 512 : (ih + 1) * 512],
                            start=(k == 0), stop=(k == 5),
                        )
                ob = osb.tile([128, L], F32R, tag="ob")
                nc.scalar.activation(ob[:], prp[:], AF.Identity,
                                     bias=pbT[:, dt_ : dt_ + 1], scale=1.0)
                nc.sync.dma_start(outD[dt_ * 128 : (dt_ + 1) * 128, :], ob[:])


        est.close()
    return nc


BEST_FLAGS = dict(gps=True, ebf16=True, allbf16=True, bskew=True, deep=False)

_BF16_KEYS = ("W_t", "Wv_t", "bv_row", "P_t", "OHOW", "RhT", "RwT",
              "ones_row")


def _convert_maps(C, xT_all, allbf16, sbf16=False):
    import ml_dtypes
    C = dict(C)
    if sbf16 and not allbf16:
        for k in ("OHOW", "RhT", "RwT"):
            C[k] = C[k].astype(ml_dtypes.bfloat16)
        return C, xT_all
    if not allbf16:
        return C, xT_all
    for k in _BF16_KEYS:
        C[k] = C[k].astype(ml_dtypes.bfloat16)
    return C, xT_all.astype(ml_dtypes.bfloat16)

_BUILT = None


def _get_built():
    global _BUILT
    if _BUILT is None:
        _BUILT = build_bass(1, **BEST_FLAGS)
    return _BUILT


def _ensure_axon():
    """Re-enable the axon backend if the caller pinned JAX_PLATFORMS=cpu
    (common in reference harnesses)."""
    import jax

    def has_axon():
        try:
            return any(getattr(d, "platform", "") == "axon" or "NC_" in str(d)
                       for d in jax.devices())
        except Exception:
            return False

    if has_axon():
        return
    os.environ.pop("JAX_PLATFORMS", None)
    try:
        jax.config.update("jax_platforms", None)
    except Exception:
        pass
    try:
        from jax._src import xla_bridge
        xla_bridge._clear_backends()
    except Exception:
        pass
    assert has_axon(), "axon/neuron devices not visible to jax"


def kernel(x, qkv_w, qkv_b, proj_w, proj_b, rel_pos_h, rel_pos_w):
    _ensure_axon()
    x = np.asarray(x, np.float32)
    B = x.shape[0]
    C = _host_prep(np.asarray(qkv_w, np.float32), np.asarray(qkv_b, np.float32),
                   np.asarray(proj_w, np.float32), np.asarray(proj_b, np.float32),
                   np.asarray(rel_pos_h, np.float32), np.asarray(rel_pos_w, np.float32))
    xT_all = np.ascontiguousarray(x.reshape(B, L, DIM).transpose(0, 2, 1))
    C, xT_all = _convert_maps(C, xT_all, BEST_FLAGS.get("allbf16", False), BEST_FLAGS.get("sbf16", False))

    nc = _get_built()
    in_maps = [dict(C, xT=xT_all[b]) for b in range(B)]
    res = run_bass_kernel_spmd(nc, in_maps, list(range(B))).results
    out = np.stack([res[b]["out"].T for b in range(B)])  # (B, 1024, 768)
    return np.ascontiguousarray(out.reshape(B, HH, WW, DIM).astype(np.float32))



# revision 45
# speedup vs baseline: 168.2591x; 168.2591x over previous
"""Trainium2 Bass kernel for nn_Attention_8083128451525 (sparse_attention).

Strategy (validated against reference on HW):
  - data-parallel: core b computes batch element b (B=8, 8 cores), no collectives
  - all matmuls bf16 (in/out), fp32 PSUM accumulation; rel err ~3e-3
  - 2D rope via stream_shuffle (pair swap) + 2 muls + 2 half-adds
  - decomposed rel-pos bias folded into ONE K=128 augmented S^T matmul:
      K~ = [roped k | onehot_h*8 | onehot_w*8],  Q~ = [roped q | U^T | V^T]
    U^T/V^T built by P = RhT.T @ rq matmuls + DRAM-bounce gather-DMA with
    overlapping access pattern; exp uses scale=1/8 (onehots pre-scaled by 8)
  - attn inner loop: S both-halves -> ONE batched exp [128,1024] -> PV,
    head-boundary software pipelining (next head's first S/exp issued
    before the previous head's last PV)
  - PSUM: s pool [128,1024]x2 shared by v/S/proj, pv 1 buf + early DVE
    copy to SBUF, mm pool for qk/rel-pos; exactly 8 banks
  - softmax denominator = ones-column in augmented V (PV row 64);
    normalization via reciprocal + DRAM-bounce 0-stride broadcast
  - proj in transposed orientation (out^T = P^T-chunks @ outt), bias as
    per-partition ACT bias, host-side transpose of the [768,1024] result
  - DMA: bulk weight/const loads on the Activation HWDGE queue, data-
    dependent per-head DMAs on SP; OHOW loaded once + per-head DVE copy;
    vaug ones-columns via memset (no template load)
"""

import os
import sys

for _p in ("/opt/trn_rl_repo", "/root/.axon_site/_ro/trn_rl_repo"):
    if os.path.isdir(_p) and _p not in sys.path:
        sys.path.insert(0, _p)

import json
from contextlib import ExitStack

import numpy as np

import bass_rust
import concourse.bass as bass
import concourse.tile as tile
from concourse import mybir
from concourse.bass_utils import run_bass_kernel_spmd

F32R = mybir.dt.float32r
F32 = mybir.dt.float32
BF16 = mybir.dt.bfloat16
AF = mybir.ActivationFunctionType

NH, HD, HH, WW = 12, 64, 32, 32
L = HH * WW          # 1024
DIM = NH * HD        # 768
ROPE_THETA = 10000.0

# ---------------------------------------------------------------------------
# BIR post-fix: hoist excess semaphore waits onto injected NoOps (walrus
# instruction structs have limited wait slots; f32r Matmult rejects >1).
# ---------------------------------------------------------------------------
_nop_counter = [0]


def _transform_bir(json_bytes: bytes) -> bytes:
    data = json.loads(json_bytes)
    for fn in data.get("functions", []):
        for blk in fn.get("blocks", []):
            out = []
            for ins in blk.get("instructions", []):
                si = ins.get("sync_info")
                waits = (si or {}).get("on_wait") or []
                if len(waits) > 1:
                    keep = waits[-1:]
                    for w in waits[:-1]:
                        _nop_counter[0] += 1
                        out.append({
                            "name": f"I-birfix-{_nop_counter[0]}",
                            "opcode": "NoOp",
                            "engine": ins.get("engine"),
                            "ins": [],
                            "outs": [],
                            "sync_info": {"on_wait": [w], "on_update": []},
                        })
                    si["on_wait"] = keep
                out.append(ins)
            blk["instructions"] = out
    return json.dumps(data).encode()


def _install_birfix():
    if getattr(bass.Bass, "_birfix_installed", False):
        return
    orig = bass.Bass.to_json_bytes

    def patched(self, *a, **kw):
        return _transform_bir(orig(self, *a, **kw))

    bass.Bass.to_json_bytes = patched
    bass.Bass._birfix_installed = True


_install_birfix()


def _vp(pairs):
    return bass_rust.VecI64Pair(pairs)


def _cap(ap, pairs, offset=None):
    ap = ap.copy()
    ap.ap = _vp(pairs)
    if offset is not None:
        ap.offset = offset
    return ap


# ---------------------------------------------------------------------------
# Host-side constant prep
# ---------------------------------------------------------------------------
def _rope_angles():
    f = 1.0 / (ROPE_THETA ** (np.arange(0, HD, 4)[: HD // 4].astype(np.float32) / HD))
    t = np.arange(L)
    tx = (t % WW).astype(np.float32)
    ty = (t // WW).astype(np.float32)
    return np.concatenate([np.outer(tx, f), np.outer(ty, f)], axis=-1)  # (L, 32)


def _host_prep(qkv_w, qkv_b, proj_w, proj_b, rel_pos_h, rel_pos_w):
    cols, bias = [], []
    for n in range(NH):
        cols.append(qkv_w[0 * DIM + n * HD : 0 * DIM + (n + 1) * HD])
        cols.append(qkv_w[1 * DIM + n * HD : 1 * DIM + (n + 1) * HD])
        bias.append(qkv_b[0 * DIM + n * HD : 0 * DIM + (n + 1) * HD])
        bias.append(qkv_b[1 * DIM + n * HD : 1 * DIM + (n + 1) * HD])
    W_t = np.ascontiguousarray(np.concatenate(cols, axis=0).T)        # (768, 1536)
    b_qk = np.concatenate(bias).reshape(NH, 128).T.copy()             # (128, 12)

    Wv_t = np.ascontiguousarray(qkv_w[2 * DIM :].T)                   # (768, 768)
    bv_row = qkv_b[2 * DIM :].reshape(1, DIM).copy()
    P_t = np.ascontiguousarray(proj_w.T)                              # (768, 768)
    pbT = np.ascontiguousarray(proj_b.reshape(6, 128).T)              # (128, 6)

    ang = _rope_angles()
    cos, sin = np.cos(ang), np.sin(ang)                               # (L, 32)
    CC = np.empty((128, L), np.float32)
    SS = np.empty((128, L), np.float32)
    for p in range(128):
        m = (p % 64) // 2
        CC[p] = cos[:, m]
        SS[p] = sin[:, m] if (p % 2) else -sin[:, m]

    j = np.arange(L)
    OHOW = np.zeros((64, L), np.float32)
    for p in range(32):
        OHOW[p] = 8.0 * ((j >> 5) == (31 - p))
        OHOW[32 + p] = 8.0 * ((j & 31) == (31 - p))

    # padded to 64 output rows (M=64) so downstream copies stay aligned
    RhT = np.zeros((HD, 64), np.float32)
    RhT[:, 0:63] = rel_pos_h.T
    RwT = np.zeros((HD, 64), np.float32)
    RwT[:, 0:63] = rel_pos_w.T

    import ml_dtypes as _mld
    CC = CC.astype(_mld.bfloat16)
    SS = SS.astype(_mld.bfloat16)
    ones_row = np.ones((1, 128), np.float32)

    return dict(W_t=W_t, b_qk=b_qk, Wv_t=Wv_t, bv_row=bv_row, P_t=P_t,
                pbT=pbT, CC=CC, SS=SS, OHOW=OHOW, RhT=RhT, RwT=RwT,
                ones_row=ones_row)


# ---------------------------------------------------------------------------
# Bass program (one core, one batch element)
# ---------------------------------------------------------------------------
def build_bass(iters=1, gps=False, ebf16=False, allbf16=False, pv2=True,
               sbf16=False, ts2=True, t22=False, deep=True, wq5=True,
               probe=(), ppse="act", ri_pool=False, pp_split=False, skew=False, psalt=False,
               bskew=False, pj_alt=False):
    probe = set(probe)
    if allbf16:
        ebf16 = True
    nc = bass.Bass()

    DT = BF16 if allbf16 else F32R
    SDT = BF16 if (sbf16 or allbf16) else F32R
    xT = nc.declare_dram_parameter("xT", [DIM, L], DT, isOutput=False)
    W_t = nc.declare_dram_parameter("W_t", [DIM, 1536], DT, isOutput=False)
    b_qk = nc.declare_dram_parameter("b_qk", [128, NH], F32, isOutput=False)
    Wv_t = nc.declare_dram_parameter("Wv_t", [DIM, DIM], DT, isOutput=False)
    bv_row = nc.declare_dram_parameter("bv_row", [1, DIM], DT, isOutput=False)
    P_t = nc.declare_dram_parameter("P_t", [DIM, DIM], DT, isOutput=False)
    pbT_d = nc.declare_dram_parameter("pbT", [128, 6], F32, isOutput=False)
    CCd = nc.declare_dram_parameter("CC", [128, L], BF16, isOutput=False)
    SSd = nc.declare_dram_parameter("SS", [128, L], BF16, isOutput=False)
    OHOWd = nc.declare_dram_parameter("OHOW", [64, L], SDT, isOutput=False)
    RhTd = nc.declare_dram_parameter("RhT", [HD, 64], SDT, isOutput=False)
    RwTd = nc.declare_dram_parameter("RwT", [HD, 64], SDT, isOutput=False)
    ones_d = nc.declare_dram_parameter("ones_row", [1, 128], DT, isOutput=False)
    outD = nc.declare_dram_parameter("out", [DIM, L], F32R, isOutput=True)

    with tile.TileContext(nc) as tc:
        est = ExitStack()
        consts = est.enter_context(tc.tile_pool(name="consts", bufs=1))
        xtp = est.enter_context(tc.tile_pool(name="xtp", bufs=1))
        wqp = est.enter_context(tc.tile_pool(name="wqp", bufs=(5 if wq5 else 4) if deep else 3))
        vwp = est.enter_context(tc.tile_pool(name="vwp", bufs=1))
        ripool = est.enter_context(tc.tile_pool(name="ripool", bufs=2))
        vap = est.enter_context(tc.tile_pool(name="vap", bufs=1))
        qap = est.enter_context(tc.tile_pool(name="qap", bufs=3))
        kap = est.enter_context(tc.tile_pool(name="kap", bufs=3))
        scr = est.enter_context(tc.tile_pool(name="scr", bufs=1))
        tsp = est.enter_context(tc.tile_pool(name="tsp", bufs=2 if ts2 else 1))
        t2p = est.enter_context(tc.tile_pool(name="t2p", bufs=2 if t22 else 1))
        pps = est.enter_context(tc.tile_pool(name="pps", bufs=2))
        epool = est.enter_context(tc.tile_pool(name="epool", bufs=4 if deep else 3))
        misc = est.enter_context(tc.tile_pool(name="misc", bufs=2))
        recp = est.enter_context(tc.tile_pool(name="recp", bufs=1))
        outtp = est.enter_context(tc.tile_pool(name="outtp", bufs=1))
        osb = est.enter_context(tc.tile_pool(name="osb", bufs=1))
        dram = est.enter_context(tc.tile_pool(name="dram", bufs=6 if ts2 else 3, space="DRAM"))
        psum = est.enter_context(tc.tile_pool(name="psum", bufs=1, space="PSUM"))
        psum_s = est.enter_context(tc.tile_pool(name="psum_s", bufs=2, space="PSUM"))
        psum_pv = est.enter_context(tc.tile_pool(name="psum_pv", bufs=1, space="PSUM"))
        pvsp = est.enter_context(tc.tile_pool(name="pvsp", bufs=2))

        # ---- constants ----
        cc = consts.tile([128, L], BF16, tag="cc")
        ss = consts.tile([128, L], BF16, tag="ss")
        bqk = consts.tile([128, NH], F32, tag="bqk")
        bv = consts.tile([1, DIM], DT, tag="bv")
        pbT = consts.tile([128, 6], F32, tag="pbT")
        rht = consts.tile([HD, 64], SDT, tag="rht")
        rwt = consts.tile([HD, 64], SDT, tag="rwt")
        ones1 = consts.tile([1, 128], DT, tag="ones1")
        ohowc = consts.tile([64, L], SDT, tag="ohowc")
        nc.scalar.dma_start(cc[:], CCd[:])
        nc.scalar.dma_start(ss[:], SSd[:])
        nc.scalar.dma_start(bqk[:], b_qk[:])
        nc.scalar.dma_start(bv[:], bv_row[:])
        nc.scalar.dma_start(pbT[:], pbT_d[:])
        nc.scalar.dma_start(rht[:], RhTd[:])
        nc.scalar.dma_start(rwt[:], RwTd[:])
        nc.scalar.dma_start(ones1[:], ones_d[:])
        nc.scalar.dma_start(ohowc[:], OHOWd[:])

        # ---- xT as one wide tile (one DMA), chunk views per k ----
        xts = []
        for k in range(6):
            t = xtp.tile([128, L], DT, tag=f"xt{k}")
            nc.scalar.dma_start(t[:], xT[k * 128 : (k + 1) * 128, :])
            xts.append(t)

        for _it in range(iters):
            # ---- per-head pipeline state (prep defined below, invoked early) ----
            outt = []
            for k in range(6):
                ot = outtp.tile([128, L], DT, tag=f"ot{k}", name=f"ot{k}")
                outt.append(ot)
            swap_mask = [i ^ 1 for i in range(32)]
            vaug = []

            # ---- per-head pipeline ----
            def prep(n):
                # qk matmul for head n: W m-tile n = [q_n | k_n] columns
                qa = qap.tile([128, L], SDT, tag="qa", name="qa")
                ka = kap.tile([128, L], SDT, tag="ka", name="ka")
                if "no_qk" not in probe:
                    wm = wqp.tile([128, 768], DT, tag="wm", name="wm")
                    nc.sync.dma_start(
                        wm[:], _cap(W_t[:], [[1536, 128], [128 * 1536, 6], [1, 128]], n * 128))
                    qkps = psum.tile([128, L], F32, tag="mm", name="qkps")
                    for k in range(6):
                        for ih in range(2):
                            nc.tensor.matmul(
                                qkps[:, ih * 512 : (ih + 1) * 512],
                                wm[:, k * 128 : (k + 1) * 128],
                                xts[k][:, ih * 512 : (ih + 1) * 512],
                                start=(k == 0), stop=(k == 5),
                            )
                    ri = ripool.tile([128, L], F32, tag="ri", name="ri")
                    if ri_pool:
                        nc.gpsimd.tensor_scalar_add(ri[:], qkps[:], bqk[:, n : n + 1])
                    else:
                        nc.scalar.activation(ri[:], qkps[:], AF.Identity,
                                             bias=bqk[:, n : n + 1], scale=1.0)

                if "no_rope" in probe or "no_qk" in probe:
                    nc.vector.tensor_copy(qa[0:64, :], cc[0:64, :])
                    nc.vector.tensor_copy(ka[0:64, :], cc[64:128, :])
                else:
                    # rope
                    ts_ = tsp.tile([128, L], F32, tag="ts", name="ts_")
                    nc.vector.stream_shuffle(ts_[:], ri[:], swap_mask)
                    t1 = scr.tile([128, L], F32, tag="t1", name="t1")
                    nc.vector.tensor_mul(t1[:], ri[:], cc[:])
                    t2 = t2p.tile([128, L], F32, tag="t2", name="t2")
                    (nc.gpsimd if gps else nc.vector).tensor_mul(t2[:], ts_[:], ss[:])
                    nc.vector.tensor_add(qa[0:64, :], t1[0:64, :], t2[0:64, :])
                    nc.vector.tensor_add(ka[0:64, :], t1[64:128, :], t2[64:128, :])
                nc.vector.tensor_copy(ka[64:128, :], ohowc[:])

                if "no_relpos" in probe:
                    nc.vector.tensor_copy(qa[64:128, :], ss[0:64, :])
                else:
                    # rel-pos P matmuls on roped q (M padded to 64);
                    # PSUM DMA'd straight to DRAM (no act copy)
                    php = psum.tile([128, L], F32, tag="mm", name="php")
                    for ih in range(2):
                        nc.tensor.matmul(php[0:64, ih * 512 : (ih + 1) * 512],
                                         rht[:], qa[0:64, ih * 512 : (ih + 1) * 512],
                                         start=True, stop=True)

                    pwp = psum.tile([128, L], F32, tag="mm", name="pwp")
                    for ih in range(2):
                        rhs = _cap(qa[0:64, :], [[L, 64], [1, 16], [32, 32]], ih * 16)
                        nc.tensor.matmul(pwp[0:64, ih * 512 : (ih + 1) * 512],
                                         rwt[:], rhs, start=True, stop=True)

                    phs = pps.tile([64, L], SDT, tag="phs", name="phs")
                    pws = pps.tile([64, L], SDT, tag="pws", name="pws")
                    if pp_split:
                        nc.vector.tensor_copy(phs[:], php[0:64, :])
                        nc.gpsimd.tensor_copy(pws[:], pwp[0:64, :])
                    else:
                        nc.scalar.activation(phs[:], php[0:64, :], AF.Copy)
                        nc.scalar.activation(pws[:], pwp[0:64, :], AF.Copy)

                    if "no_bounce" in probe:
                        nc.vector.tensor_copy(qa[64:128, :], ss[0:64, :])
                    else:
                        phd = dram.tile([64, L], SDT, tag="phd", name="phd")
                        nc.sync.dma_start(phd[:], phs[:])
                        pwd = dram.tile([64, L], SDT, tag="pwd", name="pwd")
                        nc.sync.dma_start(pwd[:], pws[:])

                        # gather U^T into qa[64:96]; gather Vt (w-major) + unpermute
                        nc.sync.dma_start(qa[64:96, :],
                                          _cap(phd[:], [[1024, 32], [1056, 32], [1, 32]]))
                        vts = misc.tile([32, L], SDT, tag="vts", name="vts")
                        nc.sync.dma_start(vts[:],
                                          _cap(pwd[:], [[1024, 32], [1056, 32], [1, 32]]))
                        (nc.gpsimd if gps else nc.vector).tensor_copy(
                            _cap(qa[96:128, :], [[L, 32], [32, 32], [1, 32]]),
                            _cap(vts[:], [[L, 32], [1, 32], [32, 32]]))
                return qa, ka

            edt = BF16 if ebf16 else F32R
            njt = 4 if "half_jt" in probe else 8

            def s_exp(n, qa, ka, jt):
                sps = psum_s.tile([128, L], F32, tag="s", name="sps")
                for ih in range(2):
                    nc.tensor.matmul(sps[:, ih * 512 : (ih + 1) * 512],
                                     ka[:, jt * 128 : (jt + 1) * 128],
                                     qa[:, ih * 512 : (ih + 1) * 512],
                                     start=True, stop=True)
                ej = epool.tile([128, L], edt, tag="ej", name="ej")
                nc.scalar.activation(ej[:], sps[:],
                                     AF.Copy if "exp_copy" in probe else AF.Exp,
                                     scale=0.125)
                return ej

            def pv_step(n, pvp, ej, jt):
                for ih in range(2):
                    nc.tensor.matmul(pvp[:, ih * 512 : (ih + 1) * 512],
                                     vaug[jt][:, 65 * n : 65 * n + 65],
                                     ej[:, ih * 512 : (ih + 1) * 512],
                                     start=(jt == 0), stop=(jt == njt - 1))

            def finish_head(n, pvp):
                # free the PV psum bank early: copy to SBUF on DVE
                pvs = pvsp.tile([65, L], F32, tag="pvs", name="pvs")
                nc.vector.tensor_copy(pvs[:], pvp[:])
                if "no_recb" in probe:
                    nc.vector.tensor_copy(
                        outt[n // 2][(n % 2) * 64 : (n % 2) * 64 + 64, :], pvs[0:64, :])
                else:
                    # normalize via reciprocal + 0-stride DMA broadcast
                    rec = recp.tile([1, L], F32, tag="rec", name="rec")
                    nc.vector.reciprocal(rec[:], pvs[64:65, :])
                    recd = dram.tile([1, L], F32, tag="recd", name="recd")
                    nc.sync.dma_start(recd[:], rec[:])
                    rec64 = misc.tile([64, L], F32, tag="rec64", name="rec64")
                    nc.sync.dma_start(rec64[:], _cap(recd[:], [[0, 64], [1, L]]))
                    nc.vector.tensor_mul(outt[n // 2][(n % 2) * 64 : (n % 2) * 64 + 64, :],
                                         pvs[0:64, :], rec64[:])

            # heads 0/1 prep first (their W loads + qk matmuls lead the queue),
            # then the v phase, whose weight DMAs trail the early wm loads
            state = {0: prep(0), 1: prep(1)}

            vws = []
            for k in range(6):
                t = vwp.tile([128, DIM], DT, tag=f"vw{k}")
                nc.scalar.dma_start(t[:], Wv_t[k * 128 : (k + 1) * 128, :])
                vws.append(t)
            for m in range(8):
                va = vap.tile([128, 780], BF16 if ebf16 else F32R, tag=f"va{m}")
                nc.vector.memset(_cap(va[:], [[780, 128], [65, 12], [1, 64]]), 0.0)
                nc.vector.memset(_cap(va[:], [[780, 128], [65, 12]], 64), 1.0)
                if "no_v" not in probe:
                    if psalt and m % 2 == 1:
                        vps = psum.tile([128, L], F32, tag="mm", name="vps")
                    else:
                        vps = psum_s.tile([128, L], F32, tag="s", name="vps")
                    for k in range(6):
                        for c0, cw in ((0, 512), (512, 256)):
                            nc.tensor.matmul(
                                vps[:, c0 : c0 + cw],
                                xts[k][:, m * 128 : (m + 1) * 128],
                                vws[k][:, c0 : c0 + cw],
                                start=(k == 0), stop=False,
                            )
                    for c0, cw in ((0, 512), (512, 256)):
                        nc.tensor.matmul(vps[:, c0 : c0 + cw], ones1[:, 0:128],
                                         bv[:, c0 : c0 + cw], start=False, stop=True)
                    nc.scalar.activation(_cap(va[:], [[780, 128], [65, 8], [1, 64]]),
                                         vps[:, 0:512], AF.Copy)
                    nc.scalar.activation(_cap(va[:], [[780, 128], [65, 4], [1, 64]], 65 * 8),
                                         vps[:, 512:768], AF.Copy)
                vaug.append(va)

            # proj weights early: vw tags are free once the v matmuls read them
            pts = []
            for k in range(6):
                t = vwp.tile([128, DIM], DT, tag=f"vw{k}")
                nc.scalar.dma_start(t[:], P_t[k * 128 : (k + 1) * 128, :])
                pts.append(t)

            if "no_attn" in probe:
                for n in range(2, NH):
                    state[n] = prep(n)
            else:
                steps = [(n, jt) for n in range(NH) for jt in range(njt)]
                ej_q = []     # (n, pvp, ej, jt) awaiting PV
                pvps = {}
                for i, (n, jt) in enumerate(steps):
                    if jt == 0:
                        if n + 2 < NH:
                            state[n + 2] = prep(n + 2)
                        pvps[n] = psum_pv.tile([65, L], F32, tag="pv", name="pvp")
                    qa, ka = state[n]
                    depth = 1 if (skew or (bskew and jt == njt - 1)) else 0
                    ej_q.append((n, pvps[n], s_exp(n, qa, ka, jt), jt))
                    if len(ej_q) > depth:
                        pn, ppvp, pej, pjt = ej_q.pop(0)
                        pv_step(pn, ppvp, pej, pjt)
                        if pjt == njt - 1:
                            finish_head(pn, ppvp)
                            state.pop(pn)
                while ej_q:
                    pn, ppvp, pej, pjt = ej_q.pop(0)
                    pv_step(pn, ppvp, pej, pjt)
                    if pjt == njt - 1:
                        finish_head(pn, ppvp)
                        state.pop(pn)

            # ---- proj (transposed orientation: out^T[d, i]) ----
            if "no_proj" in probe:
                continue
            for dt_ in range(6):
                if (psalt or pj_alt) and dt_ % 2 == 1:
                    prp = psum.tile([128, L], F32, tag="mm", name="prp")
                else:
                    prp = psum_s.tile([128, L], F32, tag="s", name="prp")
                for k in range(6):
                    for ih in range(2):
                        nc.tensor.matmul(
                            prp[:, ih * 512 : (ih + 1) * 512],
                            pts[k][:, dt_ * 128 : (dt_ + 1) * 128],
                            outt[k][:, ih * 512 : (ih + 1) * 512],
                            start=(k == 0), stop=(k == 5),
                        )
                ob = osb.tile([128, L], F32R, tag="ob")
                nc.scalar.activation(ob[:], prp[:], AF.Identity,
                                     bias=pbT[:, dt_ : dt_ + 1], scale=1.0)
                nc.sync.dma_start(outD[dt_ * 128 : (dt_ + 1) * 128, :], ob[:])


        est.close()
    return nc


BEST_FLAGS = dict(gps=True, ebf16=True, allbf16=True, bskew=True, deep=False)

_BF16_KEYS = ("W_t", "Wv_t", "bv_row", "P_t", "OHOW", "RhT", "RwT",
              "ones_row")


def _convert_maps(C, xT_all, allbf16, sbf16=False):
    import ml_dtypes
    C = dict(C)
    if sbf16 and not allbf16:
        for k in ("OHOW", "RhT", "RwT"):
            C[k] = C[k].astype(ml_dtypes.bfloat16)
        return C, xT_all
    if not allbf16:
        return C, xT_all
    for k in _BF16_KEYS:
        C[k] = C[k].astype(ml_dtypes.bfloat16)
    return C, xT_all.astype(ml_dtypes.bfloat16)

_BUILT = None


def _get_built():
    global _BUILT
    if _BUILT is None:
        _BUILT = build_bass(1, **BEST_FLAGS)
    return _BUILT


def _ensure_axon():
    """Re-enable the axon backend if the caller pinned JAX_PLATFORMS=cpu
    (common in reference harnesses)."""
    import jax

    def has_axon():
        try:
            return any(getattr(d, "platform", "") == "axon" or "NC_" in str(d)
                       for d in jax.devices())
        except Exception:
            return False

    if has_axon():
        return
    os.environ.pop("JAX_PLATFORMS", None)
    try:
        jax.config.update("jax_platforms", None)
    except Exception:
        pass
    try:
        from jax._src import xla_bridge
        xla_bridge._clear_backends()
    except Exception:
        pass
    assert has_axon(), "axon/neuron devices not visible to jax"


def kernel(x, qkv_w, qkv_b, proj_w, proj_b, rel_pos_h, rel_pos_w):
    _ensure_axon()
    x = np.asarray(x, np.float32)
    B = x.shape[0]
    C = _host_prep(np.asarray(qkv_w, np.float32), np.asarray(qkv_b, np.float32),
                   np.asarray(proj_w, np.float32), np.asarray(proj_b, np.float32),
                   np.asarray(rel_pos_h, np.float32), np.asarray(rel_pos_w, np.float32))
    xT_all = np.ascontiguousarray(x.reshape(B, L, DIM).transpose(0, 2, 1))
    C, xT_all = _convert_maps(C, xT_all, BEST_FLAGS.get("allbf16", False), BEST_FLAGS.get("sbf16", False))

    nc = _get_built()
    in_maps = [dict(C, xT=xT_all[b]) for b in range(B)]
    res = run_bass_kernel_spmd(nc, in_maps, list(range(B))).results
    out = np.stack([res[b]["out"].T for b in range(B)])  # (B, 1024, 768)
    return np.ascontiguousarray(out.reshape(B, HH, WW, DIM).astype(np.float32))

